# revision 56
# baseline (speedup 1.0000x reference)
"""Causal squeeze-excite 1d on 8 TRN2 NeuronCores.

Reference computation (per batch b):
    y = causal_ema(x)                      # y[t] = (1-a) y[t-1] + a x[t], y[0] = x[0]
    h = relu(w1 @ y[:, t] + b1)            # (32,)  per time step
    g = sigmoid(w2 @ h + b2)               # (512,) per time step
    out[:, t] = x[:, t] * g
Sharding: data-parallel over batch; core i gets x[2i:2i+2].

Structure (fp16 IO, 4-stripe PE tiling):
  - x/out/weights travel as fp16: halves HBM traffic (the kernel is
    DMA-bound at ~358 GB/s/core); fp16's 2^-11 rounding is far inside
    tolerance.  Host lays DRAM out as [128p, b, chunk, cb, t] so every
    load is 128 descriptors x 8 KB contiguous.
  - EMA commutes with the channel projection: w1 @ ema(x) == ema((a*w1) @ x),
    so the DVE scan runs on a 32-row projected sequence, not [512, T].
  - FOUR work streams stack in partition quadrants: stripe q(h,b) =
    32*(2h+b) holds (time-half h, batch b).  mm1 runs as 4 concurrent
    128x32 PE col-tiles and mm2 as 4 concurrent 32x128 row-tiles, so
    the PE array time drops ~4x -- the sigmoid stream never waits on a
    HAM-throttled matmul drain (the PE clock-gate oscillation that
    dominated run-to-run variance).
  - The scan carry crosses partition halves (rows 0-63 <-> 64-127)
    through tiny N=1 identity matmuls into PSUM: engines cannot shift
    partitions, the PE can.  fp16 identity x fp16 carry is exact.
  - b1 rides the DVE relu (fused add+max); b2 rides the sigmoid
    ACTIVATE's per-partition bias.  ACT runs sigmoids only -- it is the
    busiest compute engine; uniform 1024-col chunks minimize its
    (N+352)-cycle per-instruction overhead.
"""

import numpy as np
from contextlib import ExitStack

import concourse.bass as bass
import concourse.bacc as bacc
import concourse.tile as tile
import concourse.mybir as mybir
from concourse.bass_utils import run_bass_kernel_spmd

F32 = mybir.dt.float32
F16 = mybir.dt.float16

N_CORES = 8
B, C, T = 16, 512, 4096
CSQ = 32          # squeeze dim
P = 128           # SBUF partitions
NCB = C // P      # channel blocks (4)
B_LOC = B // N_CORES          # batches per core (2)
Tc = 1024         # time chunk
NCI = T // Tc     # DRAM chunk blocks (4)
TS = 512          # matmul / scan sub-tile (one PSUM bank)
NH = Tc // TS     # time halves per chunk (2)
PREF = 2          # load prefetch distance, in chunks
CHUNKS = [(0, 1024), (1024, 1024), (2048, 1024), (3072, 1024)]
NTH = len(CHUNKS)


def q(h, b):
    """Partition stripe base for (time-half h, batch b)."""
    return 32 * (2 * h + b)


def build_nc(B_loc, cw, C_=C, T_=T):
    assert B_loc == B_LOC
    d = 1.0 - 1.0 / cw

    nc = bacc.Bacc(trn_type="TRN2")
    xin = nc.declare_dram_parameter("x", [P, B_loc * NCI * NCB * Tc], F16,
                                    isOutput=False)
    w1e = nc.declare_dram_parameter("w1e", [P, NCB * CSQ], F16, isOutput=False)
    w2q = nc.declare_dram_parameter("w2q", [P, C_], F16, isOutput=False)
    b1q = nc.declare_dram_parameter("b1q", [P, 1], F32, isOutput=False)
    b2e = nc.declare_dram_parameter("b2e", [P, NCB], F32, isOutput=False)
    idq = nc.declare_dram_parameter("idq", [P, 64], F16, isOutput=False)
    out = nc.declare_dram_parameter("out", [P, B_loc * NCI * NCB * Tc], F16,
                                    isOutput=True)

    xv = xin.rearrange("p (b ci cb t) -> p b ci cb t", b=B_loc, ci=NCI, cb=NCB)
    ov = out.rearrange("p (b ci cb t) -> p b ci cb t", b=B_loc, ci=NCI, cb=NCB)

    with ExitStack() as ctx:
        tc = ctx.enter_context(tile.TileContext(nc))
        const = ctx.enter_context(tc.tile_pool(name="const", bufs=1))
        xpool = ctx.enter_context(
            tc.tile_pool(name="xp", bufs=2 * (PREF + 1) + 1))
        opool = ctx.enter_context(tc.tile_pool(name="op", bufs=6))
        gpool = ctx.enter_context(tc.tile_pool(name="gp", bufs=6))
        upool = ctx.enter_context(tc.tile_pool(name="up", bufs=3))
        hpool = ctx.enter_context(tc.tile_pool(name="hp", bufs=3))
        cpool = ctx.enter_context(tc.tile_pool(name="cp", bufs=2))
        php = ctx.enter_context(tc.tile_pool(name="php", bufs=2, space="PSUM"))
        pgp = ctx.enter_context(tc.tile_pool(name="pgp", bufs=2, space="PSUM"))
        xps = ctx.enter_context(tc.tile_pool(name="xps", bufs=2, space="PSUM"))

        # Consts ride the Scalar HWDGE ring so the Sync ring starts on x
        # immediately.
        w1_t = const.tile([P, NCB * CSQ], F16, tag="w1e")
        nc.scalar.dma_start(w1_t[:], w1e[:])
        w2_t = const.tile([P, C_], F16, tag="w2q")
        nc.scalar.dma_start(w2_t[:], w2q[:])
        b1_t = const.tile([P, 1], F32, tag="b1q")
        nc.scalar.dma_start(b1_t[:], b1q[:])
        b2_t = const.tile([P, NCB], F32, tag="b2e")
        nc.scalar.dma_start(b2_t[:], b2e[:])
        id_t = const.tile([P, 64], F16, tag="idq")
        nc.scalar.dma_start(id_t[:], idq[:])
        dconst = const.tile([P, TS], F32, tag="dconst")
        nc.vector.memset(dconst[:], d)

        xts = {}

        def emit_loads(ci):
            for b in range(B_loc):
                xt = xpool.tile([P, NCB * Tc], F16, tag="x", name=f"x{b}_{ci}")
                xw3 = xt[:].rearrange("p (cb t) -> p cb t", cb=NCB)
                nc.sync.dma_start(xw3[:, :, :], xv[:, b, ci, :, :])
                xts[(b, ci)] = xt

        # Chunk 0 arrives as per-batch sub-tile halves, first halves
        # leading for BOTH batches (the Sync ring is strict FIFO).
        for half in range(2):
            for b in range(B_loc):
                if half == 0:
                    xts[(b, 0)] = xpool.tile([P, NCB * Tc], F16, tag="x",
                                             name=f"x{b}_0")
                xw3 = xts[(b, 0)][:].rearrange("p (cb t) -> p cb t", cb=NCB)
                nc.sync.dma_start(
                    xw3[:, :, half * TS:(half + 1) * TS],
                    xv[:, b, 0, :, half * TS:(half + 1) * TS])
        for ci in range(1, min(PREF, NTH)):
            emit_loads(ci)

        # Warm the PE while the first x chunk is still in flight (HAM
        # grants the full clock after ~3.4us of sustained MM activity).
        scratch = php.tile([P, TS], F32, tag="ph", name="warm")
        for _ in range(10):
            nc.tensor.matmul(scratch[0:CSQ, 0:NCB * CSQ],
                             w1_t[:, 0:CSQ], w1_t[:],
                             start=True, stop=True)

        ph_pre = {}

        def phase1(ci):
            # mm1 for chunk ci: ALL four (half, batch) streams stack into
            # one PSUM tile at quadrant stripes, emission-interleaved so
            # the four 128x32 PE col-tiles co-execute.
            xws_ = [xts[(b, ci)][:].rearrange("p (cb t) -> p cb t", cb=NCB)
                    for b in range(B_loc)]
            ph = php.tile([P, TS], F32, tag="ph")
            # h-major: the half-A chains (which the scan spine needs
            # first, and whose data lands first) are never queued behind
            # half-B MMs that still wait on the second half of the load.
            for h in range(NH):
                for cb in range(NCB):
                    for b in range(B_loc):
                        s = q(h, b)
                        nc.tensor.matmul(
                            ph[s:s + CSQ, :],
                            w1_t[:, cb * CSQ:(cb + 1) * CSQ],
                            xws_[b][:, cb, h * TS:(h + 1) * TS],
                            start=(cb == 0), stop=(cb == NCB - 1),
                            tile_position=(0, s), skip_group_check=True)
            ph_pre[ci] = ph

        carryB = [None]

        def phase2(th):
            # Scan spine: time-half A lives in rows 0-63 (both batches),
            # half B in rows 64-127.  The carry crosses halves through
            # N=1 identity matmuls (PE is the only partition-shifter);
            # ut is fp16 so the identity MM operand dtypes match.
            ph = ph_pre.pop(th)
            ut = upool.tile([P, TS], F16, tag="u")
            if th == 0:
                # u_0 = cw * p_0 makes y[0] = x[0] exact.
                init = cpool.tile([P, 1], F32, tag="c")
                nc.vector.tensor_scalar_mul(
                    init[0:64, :], ph[0:64, 0:1], float(cw))
                init_lo = init[0:64, :]
            else:
                init_lo = carryB[0][0:64, 0:1]
            nc.vector.tensor_tensor_scan(
                ut[0:64, :], dconst[0:64, :], ph[0:64, :], init_lo,
                mybir.AluOpType.mult, mybir.AluOpType.add)
            cA = xps.tile([P, 1], F32, tag="cx", name="cA")
            nc.tensor.matmul(cA[64:128, 0:1], id_t[0:64, :],
                             ut[0:64, TS - 1:TS], start=True, stop=True,
                             tile_position=(0, 64))
            nc.vector.tensor_tensor_scan(
                ut[64:128, :], dconst[64:128, :], ph[64:128, :],
                cA[64:128, 0:1],
                mybir.AluOpType.mult, mybir.AluOpType.add)
            cB = xps.tile([P, 1], F32, tag="cx", name="cB")
            nc.tensor.matmul(cB[0:64, 0:1], id_t[64:128, :],
                             ut[64:128, TS - 1:TS], start=True, stop=True,
                             tile_position=(64, 0))
            carryB[0] = cB
            # Fused (u + b1) -> max(., 0) on the DVE keeps ACT free.
            ht = hpool.tile([P, TS], F16, tag="h")
            nc.vector.tensor_scalar(
                ht[:], ut[:], b1_t[:], 0.0,
                mybir.AluOpType.add, mybir.AluOpType.max)
            return ht

        phase1(0)
        for th in range(NTH):
            if th + PREF < NTH:
                emit_loads(th + PREF)
            ht = phase2(th)
            if th + 1 < NTH:
                phase1(th + 1)
            # Phase 3: mm2 + sigmoid per (b, cb); the four (half, batch)
            # streams run as four concurrent 32x128 PE row-tiles, b2
            # riding the ACTIVATE bias.
            gts = [gpool.tile([P, NCB * Tc], F16, tag="g", name=f"g{b}")
                   for b in range(B_loc)]
            gws = [g[:].rearrange("p (cb t) -> p cb t", cb=NCB) for g in gts]
            for cb in range(NCB):
                pgs = [pgp.tile([P, Tc], F32, tag="pg", name=f"pg{b}")
                       for b in range(B_loc)]
                for h in range(NH):
                    for b in range(B_loc):
                        s = q(h, b)
                        nc.tensor.matmul(
                            pgs[b][:, h * TS:(h + 1) * TS],
                            w2_t[s:s + CSQ, cb * P:(cb + 1) * P],
                            ht[s:s + CSQ, :],
                            start=True, stop=True,
                            tile_position=(s, 0))
                for b in range(B_loc):
                    nc.scalar.activation(
                        gws[b][:, cb, :], pgs[b][:],
                        mybir.ActivationFunctionType.Sigmoid,
                        bias=b2_t[:, cb:cb + 1])
            # Phase 4: gate multiply into a fresh fp16 tile (all-16-bit,
            # packed DVE rate); the final chunk goes per-cb so the very
            # last mul+store is as small as possible.  Stores stay on
            # the Sync ring with the loads.
            step = 1 if th == NTH - 1 else 2
            ots = [opool.tile([P, NCB * Tc], F16, tag="o", name=f"o{b}")
                   for b in range(B_loc)]
            ows = [o[:].rearrange("p (cb t) -> p cb t", cb=NCB) for o in ots]
            xws = [xts.pop((b, th))[:].rearrange("p (cb t) -> p cb t",
                                                 cb=NCB)
                   for b in range(B_loc)]
            # cb-outer, b-inner matches the sigmoid emission order, so
            # each mul fires right after its own sigmoid and its store
            # enters the FIFO as early as possible.
            for cbp in range(0, NCB, step):
                for b in range(B_loc):
                    nc.vector.tensor_mul(
                        ows[b][:, cbp:cbp + step, :],
                        xws[b][:, cbp:cbp + step, :],
                        gws[b][:, cbp:cbp + step, :])
                    nc.sync.dma_start(
                        ov[:, b, th, cbp:cbp + step, :],
                        ows[b][:, cbp:cbp + step, :])
    nc.compile()
    return nc


def make_in_maps(x, w1, b1, w2, b2, cw, n_cores=N_CORES):
    """Host-side shard + weight prep. Returns per-core input maps."""
    a = 1.0 / cw
    C_ = w2.shape[0]
    b_loc = x.shape[0] // n_cores

    w1sT = (np.asarray(w1) * a).T.astype(np.float32)      # [C, CSQ]
    w1e = np.empty((P, NCB * CSQ), dtype=np.float16)
    for cb in range(NCB):
        w1e[:, cb * CSQ:(cb + 1) * CSQ] = w1sT[cb * P:(cb + 1) * P, :]

    # Four stacked copies of w2^T / b1: one per (half, batch) stripe.
    w2q = np.empty((P, C_), dtype=np.float16)
    b1q = np.empty((P, 1), dtype=np.float32)
    for s in range(4):
        w2q[s * CSQ:(s + 1) * CSQ, :] = np.asarray(w2).T
        b1q[s * CSQ:(s + 1) * CSQ, 0] = np.asarray(b1)

    b2e = np.asarray(b2).astype(np.float32).reshape(NCB, P).T.copy()

    idq = np.zeros((P, 64), dtype=np.float16)
    idq[0:64, :] = np.eye(64, dtype=np.float16)
    idq[64:128, :] = np.eye(64, dtype=np.float16)

    # [B, C, T] -> per-core [P, b, ci, cb, t] fp16 (see build_nc).
    x16 = np.asarray(x).astype(np.float16)
    x16 = x16.reshape(n_cores, b_loc, NCB, P, NCI, Tc)
    x16 = np.ascontiguousarray(x16.transpose(0, 3, 1, 4, 2, 5))
    x16 = x16.reshape(n_cores, P, b_loc * NCI * NCB * Tc)

    return [
        {"x": x16[i], "w1e": w1e, "w2q": w2q, "b1q": b1q, "b2e": b2e,
         "idq": idq}
        for i in range(n_cores)
    ]


def unshard_out(results, n_cores=N_CORES, b_loc=B_LOC):
    """Per-core [P, b*ci*cb*t] fp16 -> full [B, C, T] fp32."""
    o = np.stack([r["out"] for r in results], axis=0)
    o = o.reshape(n_cores, P, b_loc, NCI, NCB, Tc)
    o = o.transpose(0, 2, 4, 1, 3, 5)          # [core, b, cb, p, ci, t]
    return np.ascontiguousarray(o).reshape(B, C, T).astype(np.float32)


_NC_CACHE = {}


def kernel(x, w1, b1, w2, b2, context_window):
    cw = int(context_window)
    x = np.asarray(x)
    key = (cw, x.shape)
    if key not in _NC_CACHE:
        _NC_CACHE[key] = build_nc(x.shape[0] // N_CORES, cw)
    nc = _NC_CACHE[key]
    in_maps = make_in_maps(
        np.asarray(x), np.asarray(w1), np.asarray(b1),
        np.asarray(w2), np.asarray(b2), cw)
    res = run_bass_kernel_spmd(nc, in_maps, core_ids=list(range(N_CORES)))
    return unshard_out(res.results)


# revision 57
# speedup vs baseline: 1.0249x; 1.0249x over previous
"""Causal squeeze-excite 1d on 8 TRN2 NeuronCores.

Reference computation (per batch b):
    y = causal_ema(x)                      # y[t] = (1-a) y[t-1] + a x[t], y[0] = x[0]
    h = relu(w1 @ y[:, t] + b1)            # (32,)  per time step
    g = sigmoid(w2 @ h + b2)               # (512,) per time step
    out[:, t] = x[:, t] * g
Sharding: data-parallel over batch; core i gets x[2i:2i+2].

Structure (fp16 IO, 4-stripe PE tiling):
  - x/out/weights travel as fp16: halves HBM traffic (the kernel is
    DMA-bound at ~358 GB/s/core); fp16's 2^-11 rounding is far inside
    tolerance.  Host lays DRAM out as [128p, b, chunk, cb, t] so every
    load is 128 descriptors x 8 KB contiguous.
  - EMA commutes with the channel projection: w1 @ ema(x) == ema((a*w1) @ x),
    so the DVE scan runs on a 32-row projected sequence, not [512, T].
  - FOUR work streams stack in partition quadrants: stripe q(h,b) =
    32*(2h+b) holds (time-half h, batch b).  mm1 runs as 4 concurrent
    128x32 PE col-tiles and mm2 as 4 concurrent 32x128 row-tiles, so
    the PE array time drops ~4x -- the sigmoid stream never waits on a
    HAM-throttled matmul drain (the PE clock-gate oscillation that
    dominated run-to-run variance).
  - The scan carry crosses partition halves (rows 0-63 <-> 64-127)
    through tiny N=1 identity matmuls into PSUM: engines cannot shift
    partitions, the PE can.  fp16 identity x fp16 carry is exact.
  - b1 rides the DVE relu (fused add+max); b2 rides the sigmoid
    ACTIVATE's per-partition bias.  ACT runs sigmoids only -- it is the
    busiest compute engine; uniform 1024-col chunks minimize its
    (N+352)-cycle per-instruction overhead.
"""

import numpy as np
from contextlib import ExitStack

import concourse.bass as bass
import concourse.bacc as bacc
import concourse.tile as tile
import concourse.mybir as mybir
from concourse.bass_utils import run_bass_kernel_spmd

F32 = mybir.dt.float32
F16 = mybir.dt.float16

N_CORES = 8
B, C, T = 16, 512, 4096
CSQ = 32          # squeeze dim
P = 128           # SBUF partitions
NCB = C // P      # channel blocks (4)
B_LOC = B // N_CORES          # batches per core (2)
Tc = 1024         # time chunk
NCI = T // Tc     # DRAM chunk blocks (4)
TS = 512          # matmul / scan sub-tile (one PSUM bank)
NH = Tc // TS     # time halves per chunk (2)
PREF = 2          # load prefetch distance, in chunks
CHUNKS = [(0, 1024), (1024, 1024), (2048, 1024), (3072, 1024)]
NTH = len(CHUNKS)


def q(h, b):
    """Partition stripe base for (time-half h, batch b)."""
    return 32 * (2 * h + b)


def build_nc(B_loc, cw, C_=C, T_=T):
    assert B_loc == B_LOC
    d = 1.0 - 1.0 / cw

    nc = bacc.Bacc(trn_type="TRN2")
    xin = nc.declare_dram_parameter("x", [P, B_loc * NCI * NCB * Tc], F16,
                                    isOutput=False)
    w1e = nc.declare_dram_parameter("w1e", [P, NCB * CSQ], F16, isOutput=False)
    w2q = nc.declare_dram_parameter("w2q", [P, C_], F16, isOutput=False)
    b1q = nc.declare_dram_parameter("b1q", [P, 1], F32, isOutput=False)
    b2e = nc.declare_dram_parameter("b2e", [P, NCB], F32, isOutput=False)
    idq = nc.declare_dram_parameter("idq", [P, 64], F16, isOutput=False)
    out = nc.declare_dram_parameter("out", [P, B_loc * NCI * NCB * Tc], F16,
                                    isOutput=True)

    xv = xin.rearrange("p (b ci cb t) -> p b ci cb t", b=B_loc, ci=NCI, cb=NCB)
    ov = out.rearrange("p (b ci cb t) -> p b ci cb t", b=B_loc, ci=NCI, cb=NCB)

    with ExitStack() as ctx:
        tc = ctx.enter_context(tile.TileContext(nc))
        const = ctx.enter_context(tc.tile_pool(name="const", bufs=1))
        xpool = ctx.enter_context(
            tc.tile_pool(name="xp", bufs=2 * (PREF + 1) + 1))
        opool = ctx.enter_context(tc.tile_pool(name="op", bufs=6))
        gpool = ctx.enter_context(tc.tile_pool(name="gp", bufs=6))
        upool = ctx.enter_context(tc.tile_pool(name="up", bufs=3))
        hpool = ctx.enter_context(tc.tile_pool(name="hp", bufs=3))
        cpool = ctx.enter_context(tc.tile_pool(name="cp", bufs=2))
        php = ctx.enter_context(tc.tile_pool(name="php", bufs=2, space="PSUM"))
        pgp = ctx.enter_context(tc.tile_pool(name="pgp", bufs=2, space="PSUM"))
        xps = ctx.enter_context(tc.tile_pool(name="xps", bufs=2, space="PSUM"))

        # Consts ride the Scalar HWDGE ring so the Sync ring starts on x
        # immediately.
        w1_t = const.tile([P, NCB * CSQ], F16, tag="w1e")
        nc.scalar.dma_start(w1_t[:], w1e[:])
        w2_t = const.tile([P, C_], F16, tag="w2q")
        nc.scalar.dma_start(w2_t[:], w2q[:])
        b1_t = const.tile([P, 1], F32, tag="b1q")
        nc.scalar.dma_start(b1_t[:], b1q[:])
        b2_t = const.tile([P, NCB], F32, tag="b2e")
        nc.scalar.dma_start(b2_t[:], b2e[:])
        id_t = const.tile([P, 64], F16, tag="idq")
        nc.scalar.dma_start(id_t[:], idq[:])
        dconst = const.tile([P, TS], F32, tag="dconst")
        nc.vector.memset(dconst[:], d)

        xts = {}

        def emit_loads(ci):
            for b in range(B_loc):
                xt = xpool.tile([P, NCB * Tc], F16, tag="x", name=f"x{b}_{ci}")
                xw3 = xt[:].rearrange("p (cb t) -> p cb t", cb=NCB)
                nc.sync.dma_start(xw3[:, :, :], xv[:, b, ci, :, :])
                xts[(b, ci)] = xt

        # Chunk 0 arrives as per-batch sub-tile halves, first halves
        # leading for BOTH batches (the Sync ring is strict FIFO).
        for half in range(2):
            for b in range(B_loc):
                if half == 0:
                    xts[(b, 0)] = xpool.tile([P, NCB * Tc], F16, tag="x",
                                             name=f"x{b}_0")
                xw3 = xts[(b, 0)][:].rearrange("p (cb t) -> p cb t", cb=NCB)
                nc.sync.dma_start(
                    xw3[:, :, half * TS:(half + 1) * TS],
                    xv[:, b, 0, :, half * TS:(half + 1) * TS])
        for ci in range(1, min(PREF, NTH)):
            emit_loads(ci)

        # Warm the PE while the first x chunk is still in flight (HAM
        # grants the full clock after ~3.4us of sustained MM activity).
        scratch = php.tile([P, TS], F32, tag="ph", name="warm")
        for _ in range(10):
            nc.tensor.matmul(scratch[0:CSQ, 0:NCB * CSQ],
                             w1_t[:, 0:CSQ], w1_t[:],
                             start=True, stop=True)

        ph_pre = {}

        def phase1(ci):
            # mm1 for chunk ci: ALL four (half, batch) streams stack into
            # one PSUM tile at quadrant stripes, emission-interleaved so
            # the four 128x32 PE col-tiles co-execute.
            xws_ = [xts[(b, ci)][:].rearrange("p (cb t) -> p cb t", cb=NCB)
                    for b in range(B_loc)]
            ph = php.tile([P, TS], F32, tag="ph")
            for cb in range(NCB):
                for h in range(NH):
                    for b in range(B_loc):
                        s = q(h, b)
                        nc.tensor.matmul(
                            ph[s:s + CSQ, :],
                            w1_t[:, cb * CSQ:(cb + 1) * CSQ],
                            xws_[b][:, cb, h * TS:(h + 1) * TS],
                            start=(cb == 0), stop=(cb == NCB - 1),
                            tile_position=(0, s), skip_group_check=True)
            ph_pre[ci] = ph

        carryB = [None]

        def phase2(th):
            # Scan spine: time-half A lives in rows 0-63 (both batches),
            # half B in rows 64-127.  The carry crosses halves through
            # N=1 identity matmuls (PE is the only partition-shifter);
            # ut is fp16 so the identity MM operand dtypes match.
            ph = ph_pre.pop(th)
            ut = upool.tile([P, TS], F16, tag="u")
            if th == 0:
                # u_0 = cw * p_0 makes y[0] = x[0] exact.
                init = cpool.tile([P, 1], F32, tag="c")
                nc.vector.tensor_scalar_mul(
                    init[0:64, :], ph[0:64, 0:1], float(cw))
                init_lo = init[0:64, :]
            else:
                init_lo = carryB[0][0:64, 0:1]
            nc.vector.tensor_tensor_scan(
                ut[0:64, :], dconst[0:64, :], ph[0:64, :], init_lo,
                mybir.AluOpType.mult, mybir.AluOpType.add)
            cA = xps.tile([P, 1], F32, tag="cx", name="cA")
            nc.tensor.matmul(cA[64:128, 0:1], id_t[0:64, :],
                             ut[0:64, TS - 1:TS], start=True, stop=True,
                             tile_position=(0, 64))
            nc.vector.tensor_tensor_scan(
                ut[64:128, :], dconst[64:128, :], ph[64:128, :],
                cA[64:128, 0:1],
                mybir.AluOpType.mult, mybir.AluOpType.add)
            cB = xps.tile([P, 1], F32, tag="cx", name="cB")
            nc.tensor.matmul(cB[0:64, 0:1], id_t[64:128, :],
                             ut[64:128, TS - 1:TS], start=True, stop=True,
                             tile_position=(64, 0))
            carryB[0] = cB
            # Fused (u + b1) -> max(., 0) on the DVE keeps ACT free.
            ht = hpool.tile([P, TS], F16, tag="h")
            nc.vector.tensor_scalar(
                ht[:], ut[:], b1_t[:], 0.0,
                mybir.AluOpType.add, mybir.AluOpType.max)
            return ht

        phase1(0)
        for th in range(NTH):
            if th + PREF < NTH:
                emit_loads(th + PREF)
            ht = phase2(th)
            if th + 1 < NTH:
                phase1(th + 1)
            # Phase 3: mm2 + sigmoid per (b, cb); the four (half, batch)
            # streams run as four concurrent 32x128 PE row-tiles, b2
            # riding the ACTIVATE bias.
            gts = [gpool.tile([P, NCB * Tc], F16, tag="g", name=f"g{b}")
                   for b in range(B_loc)]
            gws = [g[:].rearrange("p (cb t) -> p cb t", cb=NCB) for g in gts]
            for cb in range(NCB):
                pgs = [pgp.tile([P, Tc], F32, tag="pg", name=f"pg{b}")
                       for b in range(B_loc)]
                for h in range(NH):
                    for b in range(B_loc):
                        s = q(h, b)
                        nc.tensor.matmul(
                            pgs[b][:, h * TS:(h + 1) * TS],
                            w2_t[s:s + CSQ, cb * P:(cb + 1) * P],
                            ht[s:s + CSQ, :],
                            start=True, stop=True,
                            tile_position=(s, 0))
                for b in range(B_loc):
                    nc.scalar.activation(
                        gws[b][:, cb, :], pgs[b][:],
                        mybir.ActivationFunctionType.Sigmoid,
                        bias=b2_t[:, cb:cb + 1])
            # Phase 4: gate multiply into a fresh fp16 tile (all-16-bit,
            # packed DVE rate); the final chunk goes per-cb so the very
            # last mul+store is as small as possible.  Stores stay on
            # the Sync ring with the loads.
            step = 1 if th == NTH - 1 else 2
            for b in range(B_loc):
                ot = opool.tile([P, NCB * Tc], F16, tag="o", name=f"o{b}")
                ow = ot[:].rearrange("p (cb t) -> p cb t", cb=NCB)
                xw = xts.pop((b, th))[:].rearrange(
                    "p (cb t) -> p cb t", cb=NCB)
                for cbp in range(0, NCB, step):
                    nc.vector.tensor_mul(
                        ow[:, cbp:cbp + step, :],
                        xw[:, cbp:cbp + step, :],
                        gws[b][:, cbp:cbp + step, :])
                    nc.sync.dma_start(
                        ov[:, b, th, cbp:cbp + step, :],
                        ow[:, cbp:cbp + step, :])
    nc.compile()
    return nc


def make_in_maps(x, w1, b1, w2, b2, cw, n_cores=N_CORES):
    """Host-side shard + weight prep. Returns per-core input maps."""
    a = 1.0 / cw
    C_ = w2.shape[0]
    b_loc = x.shape[0] // n_cores

    w1sT = (np.asarray(w1) * a).T.astype(np.float32)      # [C, CSQ]
    w1e = np.empty((P, NCB * CSQ), dtype=np.float16)
    for cb in range(NCB):
        w1e[:, cb * CSQ:(cb + 1) * CSQ] = w1sT[cb * P:(cb + 1) * P, :]

    # Four stacked copies of w2^T / b1: one per (half, batch) stripe.
    w2q = np.empty((P, C_), dtype=np.float16)
    b1q = np.empty((P, 1), dtype=np.float32)
    for s in range(4):
        w2q[s * CSQ:(s + 1) * CSQ, :] = np.asarray(w2).T
        b1q[s * CSQ:(s + 1) * CSQ, 0] = np.asarray(b1)

    b2e = np.asarray(b2).astype(np.float32).reshape(NCB, P).T.copy()

    idq = np.zeros((P, 64), dtype=np.float16)
    idq[0:64, :] = np.eye(64, dtype=np.float16)
    idq[64:128, :] = np.eye(64, dtype=np.float16)

    # [B, C, T] -> per-core [P, b, ci, cb, t] fp16 (see build_nc).
    x16 = np.asarray(x).astype(np.float16)
    x16 = x16.reshape(n_cores, b_loc, NCB, P, NCI, Tc)
    x16 = np.ascontiguousarray(x16.transpose(0, 3, 1, 4, 2, 5))
    x16 = x16.reshape(n_cores, P, b_loc * NCI * NCB * Tc)

    return [
        {"x": x16[i], "w1e": w1e, "w2q": w2q, "b1q": b1q, "b2e": b2e,
         "idq": idq}
        for i in range(n_cores)
    ]


def unshard_out(results, n_cores=N_CORES, b_loc=B_LOC):
    """Per-core [P, b*ci*cb*t] fp16 -> full [B, C, T] fp32."""
    o = np.stack([r["out"] for r in results], axis=0)
    o = o.reshape(n_cores, P, b_loc, NCI, NCB, Tc)
    o = o.transpose(0, 2, 4, 1, 3, 5)          # [core, b, cb, p, ci, t]
    return np.ascontiguousarray(o).reshape(B, C, T).astype(np.float32)


_NC_CACHE = {}


def kernel(x, w1, b1, w2, b2, context_window):
    cw = int(context_window)
    x = np.asarray(x)
    key = (cw, x.shape)
    if key not in _NC_CACHE:
        _NC_CACHE[key] = build_nc(x.shape[0] // N_CORES, cw)
    nc = _NC_CACHE[key]
    in_maps = make_in_maps(
        np.asarray(x), np.asarray(w1), np.asarray(b1),
        np.asarray(w2), np.asarray(b2), cw)
    res = run_bass_kernel_spmd(nc, in_maps, core_ids=list(range(N_CORES)))
    return unshard_out(res.results)
